# revision 14
# baseline (speedup 1.0000x reference)
"""RBF feature kernel for TRN2: out[n, m] = exp(-||x_n - r_m||^2).

Strategy (8-core data parallel, x sharded along N, r replicated):
  dist2 = ||x||^2 + ||r||^2 - 2 x.r.  The GEMM runs in fp16 at 1-pass PE
  rate (f32 needs 4 passes) using an error-compensated hi/lo split:
     x ~= xh + xl,   2r ~= sh + sl   (fp16 hi + fp16 residual)
  PSUM accumulates two chained matmuls per output chunk:
     mm1 (K=128): [xh; xl]^T x [sh; sh]  = (xh+xl).sh
     mm2 (K=66):  [xh; 1; 1]^T x [sl; -rsq_hi; -rsq_lo]
                                         = xh.sl - r_sq
  (the dropped xl.sl term is ~2^-22 — negligible), i.e. PSUM ~= 2 x.r - r_sq
  to ~1e-4 absolute.  ScalarE then applies exp with the per-partition bias
  -||x||^2 kept in f32:  out = exp(PSUM - x_sq) = exp(-dist2), written
  directly as bf16 (halves the output-store DMA; outputs are ~1e-13..1e-18
  so bf16's f32-sized exponent range is required — fp16 would flush to 0).
  Per 128-row x tile: 8 matmuls (4 chunks x 2) into PSUM, 4 exp ops, one
  0.5 MB bf16 store.
"""

import numpy as np

import concourse.bass as bass
import concourse.tile as tile
from concourse import mybir
from concourse.bass_utils import run_bass_kernel_spmd
from concourse.masks import make_identity
from concourse.tile import add_dep_helper

F32 = mybir.dt.float32
F16 = mybir.dt.float16
# Output is stored bf16 (halves the dominant output-store DMA traffic) and
# upcast to f32 host-side. bf16 keeps f32's exponent range — required, since
# outputs are ~1e-13..1e-18 (fp16 would flush them to zero); bf16 rounding is
# ~0.2% relative, far under the 2e-2 gate.
OUT_DT = mybir.dt.bfloat16

N, D = 32768, 64
M = 2048
NCORES = 8
NSHARD = N // NCORES          # 4096 rows of x per core
P = 128
KAUG = D + 2                  # 66
NT = NSHARD // P              # 32 x tiles per core
MT = M // P                   # 16 r tiles
NCHUNK = 512                  # matmul free dim (one PSUM bank)
NJ = M // NCHUNK              # 4

_NC_CACHE = {}


def _build_nc() -> bass.Bass:
    nc = bass.Bass()

    x = nc.declare_dram_parameter("x", [NSHARD, D], F32, isOutput=False)
    r = nc.declare_dram_parameter("reference_points", [M, D], F32, isOutput=False)
    out = nc.declare_dram_parameter("out", [NSHARD, M], OUT_DT, isOutput=True)

    with tile.TileContext(nc) as tc:
        with (
            tc.tile_pool(name="singles", bufs=1) as singles,
            tc.tile_pool(name="outs", bufs=4) as out_pool,
        ):
            # Walrus codegen allows only ONE sync wait per Matmult, so every
            # matmul input must come from a single proc (DVE): the gpsimd-built
            # identity is bounced through a DVE copy, and DMA-loaded data is
            # staged in big tiles that per-tile DVE copies read from.
            ident_g = singles.tile([P, P], F32)
            make_identity(nc, ident_g)
            ident = singles.tile([P, P], F32)
            nc.vector.tensor_copy(ident, ident_g)
            ident16 = singles.tile([P, P], F16)
            nc.vector.tensor_copy(ident16, ident_g)

            # Staging layout ("(p t)" / "(t p)" permutes keep every DMA run
            # contiguous per partition — see store AP below which un-permutes):
            #   r_sb[:, t, :]: cols 0..63 = 2*r, col 64 = -r_sq rounded-to-fp16
            #   residual target col 65 = -r_sq - fp16(-r_sq)   (lo part)
            #   x_sb[:, i, :]: cols 0..63 = x
            # The hi/lo split of -r_sq is computed in column layout BEFORE the
            # transpose (DVE partition bases must be 0/32/64/96, so rows 64/65
            # of sT cannot be assembled row-by-row after the transpose).
            # Loads go on the ACT HWDGE ring so they never queue behind
            # output stores on the SP ring.
            r_sb = singles.tile([P, MT, KAUG], F32)
            nc.scalar.dma_start(
                out=r_sb[:, :, 0:D], in_=r.rearrange("(t p) k -> p t k", p=P)
            )
            x_sb = singles.tile([P, NT, D], F32)
            nc.scalar.dma_start(
                out=x_sb, in_=x.rearrange("(p t) k -> p t k", p=P)
            )
            out_pt = out.rearrange("(p t) m -> p t m", p=P)
            rsq_s = singles.tile([P, MT, D], F32)
            nc.vector.tensor_mul(rsq_s, r_sb[:, :, 0:D], r_sb[:, :, 0:D])
            nc.vector.reduce_sum(
                out=r_sb[:, :, D : D + 1], in_=rsq_s, axis=mybir.AxisListType.X
            )
            nc.vector.tensor_scalar_mul(
                r_sb[:, :, D : D + 1], r_sb[:, :, D : D + 1], -1.0
            )
            nrs16 = singles.tile([P, MT, 1], F16)
            nc.vector.tensor_copy(nrs16, r_sb[:, :, D : D + 1])
            nrs32 = singles.tile([P, MT, 1], F32)
            nc.vector.tensor_copy(nrs32, nrs16)
            nc.vector.tensor_sub(
                r_sb[:, :, D + 1 : D + 2], r_sb[:, :, D : D + 1], nrs32
            )
            nc.vector.tensor_scalar_mul(r_sb[:, :, 0:D], r_sb[:, :, 0:D], 2.0)

            # sT[k, m] f32: rows 0..63 = (2r)^T, row 64 = -r_sq, row 65 = lo
            sT = singles.tile([KAUG, M], F32)
            # fp16 rhs pair: rhs_h rows 0..63 = sh, rows 64..127 = sh (dup);
            # rhs_b rows 0..63 = sl, row 64 = -rsq_hi, row 65 = -rsq_lo
            rhs_h = singles.tile([P, M], F16)
            rhs_b = singles.tile([KAUG, M], F16)
            # fp16 lhsT pair: lhsT_a rows 0..63 = xh^T, rows 64..127 = xl^T;
            # lhsT_b rows 0..63 = xh^T, rows 64..65 = ones
            lhsT_a = singles.tile([P, NSHARD], F16)
            lhsT_b = singles.tile([KAUG, NSHARD], F16)

            # ps_tr (transposes) and ps_mm (matmul accumulators) are disjoint
            # PSUM pools open simultaneously (2 + 6 banks): no pool-release
            # boundary, so transposes, matmuls, exp and stores all pipeline.
            with (
                tc.tile_pool(name="ps_tr", bufs=2, space="PSUM") as ps_tr,
                tc.tile_pool(name="ps_mm", bufs=6, space="PSUM") as ps_mm,
            ):
                # ---- transposed r side first: every matmul needs it, so it
                # must be ready before the pipeline ramps ----
                for t in range(MT):
                    ps = ps_tr.tile([KAUG, P], F32)
                    nc.tensor.transpose(ps, r_sb[:, t, :], ident)
                    nc.vector.tensor_copy(sT[:, t * P : (t + 1) * P], ps)
                # hi/lo split of the fp16 rhs, whole-row DVE ops
                nc.vector.tensor_copy(rhs_h[0:D, :], sT[0:D, :])
                nc.vector.tensor_copy(rhs_h[D : 2 * D, :], sT[0:D, :])
                sh32 = singles.tile([D, M], F32)
                nc.vector.tensor_copy(sh32, rhs_h[0:D, :])
                sl32 = singles.tile([D, M], F32)
                nc.vector.tensor_sub(sl32, sT[0:D, :], sh32)
                nc.vector.tensor_copy(rhs_b[0:D, :], sl32)
                # rows 64/65 (= -rsq_hi, -rsq_lo) in one aligned 2-partition
                # copy; the fp16 cast on write reproduces the nrs16 rounding
                nc.vector.tensor_copy(rhs_b[D : D + 2, :], sT[D : D + 2, :])
                nc.vector.memset(lhsT_b[D : D + 2, :], 1.0)

                # x prep: -||x||^2 bias (f32) + fp16 hi/lo split, chunked so
                # no single DVE op head-of-line-blocks the small lhsT copies
                xsq_nb = singles.tile([P, NT, 1], F32)
                xsq_s = singles.tile([P, NT, D], F32)
                xh16 = singles.tile([P, NT, D], F16)
                xh32 = singles.tile([P, NT, D], F32)
                xl16 = singles.tile([P, NT, D], F16)
                XC = 4
                for g in range(0, NT, XC):
                    sl = slice(g, g + XC)
                    nc.vector.tensor_mul(
                        xsq_s[:, sl, :], x_sb[:, sl, :], x_sb[:, sl, :]
                    )
                    nc.vector.reduce_sum(
                        out=xsq_nb[:, sl, :],
                        in_=xsq_s[:, sl, :],
                        axis=mybir.AxisListType.X,
                    )
                    nc.vector.tensor_scalar_mul(
                        xsq_nb[:, sl, :], xsq_nb[:, sl, :], -1.0
                    )
                    nc.vector.tensor_copy(xh16[:, sl, :], x_sb[:, sl, :])
                    nc.vector.tensor_copy(xh32[:, sl, :], xh16[:, sl, :])
                    nc.vector.tensor_sub(
                        xl16[:, sl, :], x_sb[:, sl, :], xh32[:, sl, :]
                    )

                # ---- main loop, software-staggered: transpose tile i+1 on PE
                # while DVE copies tile i's lhsT out of PSUM, so the matmuls
                # (and therefore exp + stores) start early ----
                def x_transpose(i):
                    blk = slice(i * P, (i + 1) * P)
                    ps_h = ps_tr.tile([D, P], F16, name="ps_xh", tag="ps")
                    nc.tensor.transpose(ps_h, xh16[:, i, :], ident16)
                    nc.vector.tensor_copy(lhsT_a[0:D, blk], ps_h)
                    nc.vector.tensor_copy(lhsT_b[0:D, blk], ps_h)
                    ps_l = ps_tr.tile([D, P], F16, name="ps_xl", tag="ps")
                    nc.tensor.transpose(ps_l, xl16[:, i, :], ident16)
                    nc.vector.tensor_copy(lhsT_a[D : 2 * D, blk], ps_l)

                x_transpose(0)
                for i in range(NT):
                    if i + 1 < NT:
                        x_transpose(i + 1)
                    blk = slice(i * P, (i + 1) * P)
                    ot = out_pool.tile([P, M], OUT_DT)
                    for j in range(NJ):
                        ch = slice(j * NCHUNK, (j + 1) * NCHUNK)
                        pm = ps_mm.tile([P, NCHUNK], F32)
                        nc.tensor.matmul(
                            pm, lhsT_a[:, blk], rhs_h[:, ch],
                            start=True, stop=False,
                        )
                        nc.tensor.matmul(
                            pm, lhsT_b[:, blk], rhs_b[:, ch],
                            start=False, stop=True,
                        )
                        # one activation per PSUM bank: a multi-bank PSUM AP
                        # in a single ACT op faults the exec unit
                        nc.scalar.activation(
                            ot[:, ch],
                            pm,
                            mybir.ActivationFunctionType.Exp,
                            bias=xsq_nb[:, i, :],
                            scale=1.0,
                        )
                    nc.sync.dma_start(out=out_pt[:, i, :], in_=ot)

    _elide_transitive_matmul_waits(nc)
    return nc


def _elide_transitive_matmul_waits(nc) -> None:
    """Walrus codegen accepts at most ONE sync wait per Matmult instruction.

    Tile's semaphore assignment is not transitively minimal across procs: a
    matmul that waits on both ACT (PSUM WAR) and PE (PSUM WAW) keeps the PE
    wait even though the ACT reader it waits on had itself waited on a higher
    PE tick.  This pass computes happens-before vector clocks over the
    scheduled program and drops any Matmult wait that is implied by the
    instruction's program-order context plus its other waits.  It raises if a
    Matmult still carries more than one wait afterwards (compile would fail).
    """
    import bisect
    from collections import defaultdict

    all_insts = []
    for f in nc.m.functions:
        for bb in f.blocks:
            all_insts.extend(bb.instructions)

    # cumulative semaphore values in scheduled order; producers[s] maps the
    # running total to the instruction index whose completion reaches it
    cum = defaultdict(int)
    producers = defaultdict(list)  # sem id -> [(cum_value, idx)]
    inst_updates = defaultdict(list)  # idx -> [(sem, cum_value_at_completion)]
    poisoned = set()
    for idx, inst in enumerate(all_insts):
        si = inst.sync_info
        if si is None:
            continue
        for u in si.on_update or []:
            if u.update_mode != "sem-inc" or u.update_reg is not None:
                poisoned.add(u.id)
                continue
            cum[u.id] += u.update_value
            producers[u.id].append((cum[u.id], idx))
            inst_updates[idx].append((u.id, cum[u.id]))

    def producer_idx(sem, val):
        lst = producers.get(sem)
        if not lst:
            return None
        vals = [c for c, _ in lst]
        i = bisect.bisect_left(vals, val)
        return lst[i][1] if i < len(vals) else None

    prev_idx = [None] * len(all_insts)
    last_on_engine = {}
    for idx, inst in enumerate(all_insts):
        e = inst.engine
        prev_idx[idx] = last_on_engine.get(e)
        last_on_engine[e] = idx

    def usable_waits(inst):
        si = inst.sync_info
        if si is None:
            return []
        return [
            w
            for w in si.on_wait or []
            if w.sync_type == "semaphore"
            and w.wait_mode == "sem-ge-imm"
            and w.wait_reg is None
            and w.id not in poisoned
        ]

    issue_vc = {}
    comp_vc = {}

    def merge(dst, src):
        for k, v in src.items():
            if dst.get(k, -1) < v:
                dst[k] = v

    def is_async_dispatch(inst):
        # HWDGE/SWDGE DMA waits are enforced in the DMA ring, not by the
        # issuing sequencer — successors on the same engine do NOT
        # happen-after them, so their waits must not leak into program-order
        # context.
        return type(inst).__name__ in ("InstDMACopy", "InstTriggeredCopy")

    def get_issue_vc(idx):
        # context known when instruction idx ISSUES: previous same-engine
        # instruction's issue context (NOT its async updates) plus the
        # producers' completion clocks of this instruction's own waits
        if idx in issue_vc:
            return issue_vc[idx]
        issue_vc[idx] = {}  # cycle guard: conservative empty
        vc = {}
        p = prev_idx[idx]
        while p is not None and is_async_dispatch(all_insts[p]):
            p = prev_idx[p]  # skip async-dispatch instructions' contexts
        if p is not None:
            merge(vc, get_issue_vc(p))
        for w in usable_waits(all_insts[idx]):
            pi = producer_idx(w.id, w.wait_value)
            if pi is not None:
                merge(vc, get_comp_vc(pi))
            if vc.get(w.id, -1) < w.wait_value:
                vc[w.id] = w.wait_value
        issue_vc[idx] = vc
        return vc

    def get_comp_vc(idx):
        if idx in comp_vc:
            return comp_vc[idx]
        comp_vc[idx] = {}  # cycle guard
        vc = dict(get_issue_vc(idx))
        for sem, val in inst_updates.get(idx, []):
            if vc.get(sem, -1) < val:
                vc[sem] = val
        comp_vc[idx] = vc
        return vc

    stripped = 0
    for idx, inst in enumerate(all_insts):
        si = inst.sync_info
        waits = list(si.on_wait or []) if si else []
        if len(waits) <= 1:
            continue
        keep = list(waits)
        for w in list(keep):
            if (
                w.sync_type != "semaphore"
                or w.wait_mode != "sem-ge-imm"
                or w.wait_reg is not None
                or w.id in poisoned
            ):
                continue
            ctx = {}
            p = prev_idx[idx]
            if p is not None:
                merge(ctx, get_issue_vc(p))
            for w2 in keep:
                if w2 is w:
                    continue
                pi = producer_idx(w2.id, w2.wait_value)
                if pi is not None:
                    merge(ctx, get_comp_vc(pi))
                if ctx.get(w2.id, -1) < w2.wait_value:
                    ctx[w2.id] = w2.wait_value
            if ctx.get(w.id, -1) >= w.wait_value:
                keep.remove(w)
                stripped += 1
        if len(keep) != len(waits):
            si.on_wait = keep

    # TPB compute instructions encode exactly ONE wait slot (one
    # NEURON_ISA_TPB_EVENTS field per struct); only DMA instructions may carry
    # more.  Split any surviving extra waits into standalone EventSemaphore
    # instructions on the same engine queue immediately before the owner.
    ev_n = 0
    for f in nc.m.functions:
        for bb in f.blocks:
            insts = list(bb.instructions)
            out = []
            changed = False
            for inst in insts:
                si = inst.sync_info
                waits = list(si.on_wait or []) if si else []
                if len(waits) > 1:
                    for w in waits[:-1]:
                        ev_n += 1
                        ev = mybir.InstEventSemaphore(
                            name=f"evsplit-{ev_n}",
                            engine=inst.engine,
                            sync_info=mybir.SyncInfo(on_wait=[w], on_update=[]),
                        )
                        out.append(ev)
                        changed = True
                    si.on_wait = [waits[-1]]
                out.append(inst)
            if changed:
                bb.instructions = out


def _get_nc() -> bass.Bass:
    if "nc" not in _NC_CACHE:
        _NC_CACHE["nc"] = _build_nc()
    return _NC_CACHE["nc"]


def kernel(x: np.ndarray, reference_points: np.ndarray) -> np.ndarray:
    x = np.ascontiguousarray(x, dtype=np.float32)
    r = np.ascontiguousarray(reference_points, dtype=np.float32)
    assert x.shape == (N, D) and r.shape == (M, D)

    nc = _get_nc()
    in_maps = [
        {"x": x[c * NSHARD : (c + 1) * NSHARD], "reference_points": r}
        for c in range(NCORES)
    ]
    # The exec unit is occasionally found wedged (NRT_EXEC_UNIT_UNRECOVERABLE,
    # left over from an earlier process); the terminal auto-recovers it after
    # ~1-3 minutes, so retry with backoff long enough to cover that window.
    import time as _time

    last = None
    for backoff in (30.0, 90.0, 180.0, None):
        try:
            res = run_bass_kernel_spmd(nc, in_maps, list(range(NCORES)))
            break
        except Exception as e:  # noqa: BLE001 - device-transient errors
            last = e
            if backoff is None:
                raise
            _time.sleep(backoff)
    else:
        raise last
    full = np.concatenate([res.results[c]["out"] for c in range(NCORES)], axis=0)
    return full.astype(np.float32)



# revision 17
# speedup vs baseline: 1.1672x; 1.1672x over previous
"""RBF feature kernel for TRN2: out[n, m] = exp(-||x_n - r_m||^2).

Strategy (8-core data parallel, x sharded along N, r replicated):
  dist2 = ||x||^2 + ||r||^2 - 2 x.r.  The GEMM runs in fp16 at 1-pass PE
  rate (f32 needs 4 passes) using an error-compensated hi/lo split:
     x ~= xh + xl,   2r ~= sh + sl   (fp16 hi + fp16 residual)
  PSUM accumulates two chained matmuls per output chunk:
     mm1 (K=128): [xh; xl]^T x [sh; sh]  = (xh+xl).sh
     mm2 (K=66):  [xh; 1; 1]^T x [sl; -rsq_hi; -rsq_lo]
                                         = xh.sl - r_sq
  (the dropped xl.sl term is ~2^-22 — negligible), i.e. PSUM ~= 2 x.r - r_sq
  to ~1e-4 absolute.  ScalarE then applies exp with the per-partition bias
  -||x||^2 kept in f32:  out = exp(PSUM - x_sq) = exp(-dist2), written
  directly as bf16 (halves the output-store DMA; outputs are ~1e-13..1e-18
  so bf16's f32-sized exponent range is required — fp16 would flush to 0).
  Per 128-row x tile: 8 matmuls (4 chunks x 2) into PSUM, 4 exp ops, one
  0.5 MB bf16 store.
"""

import numpy as np

import concourse.bass as bass
import concourse.tile as tile
from concourse import mybir
from concourse.bass_utils import run_bass_kernel_spmd
from concourse.masks import make_identity
from concourse.tile import add_dep_helper

F32 = mybir.dt.float32
F16 = mybir.dt.float16
# Output is stored bf16 (halves the dominant output-store DMA traffic) and
# upcast to f32 host-side. bf16 keeps f32's exponent range — required, since
# outputs are ~1e-13..1e-18 (fp16 would flush them to zero); bf16 rounding is
# ~0.2% relative, far under the 2e-2 gate.
OUT_DT = mybir.dt.bfloat16

N, D = 32768, 64
M = 2048
NCORES = 8
NSHARD = N // NCORES          # 4096 rows of x per core
P = 128
KAUG = D + 2                  # 66
NT = NSHARD // P              # 32 x tiles per core
MT = M // P                   # 16 r tiles
NCHUNK = 512                  # matmul free dim (one PSUM bank)
NJ = M // NCHUNK              # 4

_NC_CACHE = {}


def _build_nc() -> bass.Bass:
    nc = bass.Bass()

    x = nc.declare_dram_parameter("x", [NSHARD, D], F32, isOutput=False)
    r = nc.declare_dram_parameter("reference_points", [M, D], F32, isOutput=False)
    out = nc.declare_dram_parameter("out", [NSHARD, M], OUT_DT, isOutput=True)

    with tile.TileContext(nc) as tc:
        with (
            tc.tile_pool(name="singles", bufs=1) as singles,
            tc.tile_pool(name="outs", bufs=4) as out_pool,
        ):
            # Walrus codegen allows only ONE sync wait per Matmult, so every
            # matmul input must come from a single proc (DVE): the gpsimd-built
            # identity is bounced through a DVE copy, and DMA-loaded data is
            # staged in big tiles that per-tile DVE copies read from.
            ident_g = singles.tile([P, P], F32)
            make_identity(nc, ident_g)
            ident = singles.tile([P, P], F32)
            nc.vector.tensor_copy(ident, ident_g)
            ident16 = singles.tile([P, P], F16)
            nc.vector.tensor_copy(ident16, ident_g)

            # Staging layout ("(p t)" / "(t p)" permutes keep every DMA run
            # contiguous per partition — see store AP below which un-permutes):
            #   r_sb[:, t, :]: cols 0..63 = 2*r, col 64 = -r_sq rounded-to-fp16
            #   residual target col 65 = -r_sq - fp16(-r_sq)   (lo part)
            #   x_sb[:, i, :]: cols 0..63 = x
            # The hi/lo split of -r_sq is computed in column layout BEFORE the
            # transpose (DVE partition bases must be 0/32/64/96, so rows 64/65
            # of sT cannot be assembled row-by-row after the transpose).
            # Loads go on the ACT HWDGE ring so they never queue behind
            # output stores on the SP ring.
            r_sb = singles.tile([P, MT, KAUG], F32)
            nc.scalar.dma_start(
                out=r_sb[:, :, 0:D], in_=r.rearrange("(t p) k -> p t k", p=P)
            )
            x_sb = singles.tile([P, NT, D], F32)
            nc.scalar.dma_start(
                out=x_sb, in_=x.rearrange("(p t) k -> p t k", p=P)
            )
            out_pt = out.rearrange("(p t) m -> p t m", p=P)
            rsq_s = singles.tile([P, MT, D], F32)
            nc.vector.tensor_mul(rsq_s, r_sb[:, :, 0:D], r_sb[:, :, 0:D])
            nc.vector.reduce_sum(
                out=r_sb[:, :, D : D + 1], in_=rsq_s, axis=mybir.AxisListType.X
            )
            nc.vector.tensor_scalar_mul(
                r_sb[:, :, D : D + 1], r_sb[:, :, D : D + 1], -1.0
            )
            nrs16 = singles.tile([P, MT, 1], F16)
            nc.vector.tensor_copy(nrs16, r_sb[:, :, D : D + 1])
            nrs32 = singles.tile([P, MT, 1], F32)
            nc.vector.tensor_copy(nrs32, nrs16)
            nc.vector.tensor_sub(
                r_sb[:, :, D + 1 : D + 2], r_sb[:, :, D : D + 1], nrs32
            )
            nc.vector.tensor_scalar_mul(r_sb[:, :, 0:D], r_sb[:, :, 0:D], 2.0)

            # sT[k, m] f32: rows 0..63 = (2r)^T, row 64 = -r_sq, row 65 = lo
            sT = singles.tile([KAUG, M], F32)
            # fp16 rhs pair: rhs_h rows 0..63 = sh, rows 64..127 = sh (dup);
            # rhs_b rows 0..63 = sl, row 64 = -rsq_hi, row 65 = -rsq_lo
            rhs_h = singles.tile([P, M], F16)
            rhs_b = singles.tile([KAUG, M], F16)
            # fp16 lhsT pair: lhsT_a rows 0..63 = xh^T, rows 64..127 = xl^T;
            # lhsT_b rows 0..63 = xh^T, rows 64..65 = ones
            lhsT_a = singles.tile([P, NSHARD], F16)
            lhsT_b = singles.tile([KAUG, NSHARD], F16)

            # ps_tr (transposes) and ps_mm (matmul accumulators) are disjoint
            # PSUM pools open simultaneously (2 + 6 banks): no pool-release
            # boundary, so transposes, matmuls, exp and stores all pipeline.
            with (
                tc.tile_pool(name="ps_tr", bufs=2, space="PSUM") as ps_tr,
                tc.tile_pool(name="ps_mm", bufs=6, space="PSUM") as ps_mm,
            ):
                # ---- transposed r side first: every matmul needs it, so it
                # must be ready before the pipeline ramps ----
                # r transposes + rhs fp16 split, chunked by 512-col output
                # chunk so matmul j=0 of tile 0 only waits on the first 4 r
                # tiles + one chunk of split ops (the DVE queue is in-order:
                # whole-row splits would push the first matmul ~15us out)
                sh32 = singles.tile([D, M], F32)
                sl32 = singles.tile([D, M], F32)
                nc.vector.memset(lhsT_b[D : D + 2, :], 1.0)
                TPC = NCHUNK // P  # r tiles per 512-col chunk
                for cc in range(NJ):
                    for t in range(cc * TPC, (cc + 1) * TPC):
                        ps = ps_tr.tile([KAUG, P], F32)
                        nc.tensor.transpose(ps, r_sb[:, t, :], ident)
                        nc.vector.tensor_copy(sT[:, t * P : (t + 1) * P], ps)
                    cs = slice(cc * NCHUNK, (cc + 1) * NCHUNK)
                    nc.vector.tensor_copy(rhs_h[0:D, cs], sT[0:D, cs])
                    nc.vector.tensor_copy(rhs_h[D : 2 * D, cs], sT[0:D, cs])
                    nc.vector.tensor_copy(sh32[:, cs], rhs_h[0:D, cs])
                    nc.vector.tensor_sub(sl32[:, cs], sT[0:D, cs], sh32[:, cs])
                    nc.vector.tensor_copy(rhs_b[0:D, cs], sl32[:, cs])
                    # rows 64/65 (= -rsq_hi, -rsq_lo) in one aligned
                    # 2-partition copy; the fp16 cast on write reproduces the
                    # nrs16 rounding
                    nc.vector.tensor_copy(rhs_b[D : D + 2, cs], sT[D : D + 2, cs])

                # x prep: -||x||^2 bias (f32) + fp16 hi/lo split. Chunk 0 is
                # emitted here; later chunks stream inside the main loop so
                # they don't head-of-line-block tile 0's lhsT copies
                xsq_nb = singles.tile([P, NT, 1], F32)
                xsq_s = singles.tile([P, NT, D], F32)
                xh16 = singles.tile([P, NT, D], F16)
                xh32 = singles.tile([P, NT, D], F32)
                xl16 = singles.tile([P, NT, D], F16)
                XC = 4

                def x_prep(g):
                    sl = slice(g, g + XC)
                    nc.vector.tensor_mul(
                        xsq_s[:, sl, :], x_sb[:, sl, :], x_sb[:, sl, :]
                    )
                    nc.vector.reduce_sum(
                        out=xsq_nb[:, sl, :],
                        in_=xsq_s[:, sl, :],
                        axis=mybir.AxisListType.X,
                    )
                    nc.vector.tensor_scalar_mul(
                        xsq_nb[:, sl, :], xsq_nb[:, sl, :], -1.0
                    )
                    nc.vector.tensor_copy(xh16[:, sl, :], x_sb[:, sl, :])
                    nc.vector.tensor_copy(xh32[:, sl, :], xh16[:, sl, :])
                    nc.vector.tensor_sub(
                        xl16[:, sl, :], x_sb[:, sl, :], xh32[:, sl, :]
                    )

                x_prep(0)

                # ---- main loop, software-staggered: transpose tile i+1 on PE
                # while DVE copies tile i's lhsT out of PSUM, so the matmuls
                # (and therefore exp + stores) start early ----
                def x_transpose(i):
                    blk = slice(i * P, (i + 1) * P)
                    ps_h = ps_tr.tile([D, P], F16, name="ps_xh", tag="ps")
                    nc.tensor.transpose(ps_h, xh16[:, i, :], ident16)
                    nc.vector.tensor_copy(lhsT_a[0:D, blk], ps_h)
                    nc.vector.tensor_copy(lhsT_b[0:D, blk], ps_h)
                    ps_l = ps_tr.tile([D, P], F16, name="ps_xl", tag="ps")
                    nc.tensor.transpose(ps_l, xl16[:, i, :], ident16)
                    nc.vector.tensor_copy(lhsT_a[D : 2 * D, blk], ps_l)

                x_transpose(0)
                for i in range(NT):
                    if i % XC == 0 and i + XC < NT:
                        x_prep(i + XC)  # stream the next x chunk ahead
                    if i + 1 < NT:
                        x_transpose(i + 1)
                    blk = slice(i * P, (i + 1) * P)
                    ot = out_pool.tile([P, M], OUT_DT)
                    for j in range(NJ):
                        ch = slice(j * NCHUNK, (j + 1) * NCHUNK)
                        pm = ps_mm.tile([P, NCHUNK], F32)
                        nc.tensor.matmul(
                            pm, lhsT_a[:, blk], rhs_h[:, ch],
                            start=True, stop=False,
                        )
                        nc.tensor.matmul(
                            pm, lhsT_b[:, blk], rhs_b[:, ch],
                            start=False, stop=True,
                        )
                        # one activation per PSUM bank: a multi-bank PSUM AP
                        # in a single ACT op faults the exec unit
                        nc.scalar.activation(
                            ot[:, ch],
                            pm,
                            mybir.ActivationFunctionType.Exp,
                            bias=xsq_nb[:, i, :],
                            scale=1.0,
                        )
                    nc.sync.dma_start(out=out_pt[:, i, :], in_=ot)

    _elide_transitive_matmul_waits(nc)
    return nc


def _elide_transitive_matmul_waits(nc) -> None:
    """Walrus codegen accepts at most ONE sync wait per Matmult instruction.

    Tile's semaphore assignment is not transitively minimal across procs: a
    matmul that waits on both ACT (PSUM WAR) and PE (PSUM WAW) keeps the PE
    wait even though the ACT reader it waits on had itself waited on a higher
    PE tick.  This pass computes happens-before vector clocks over the
    scheduled program and drops any Matmult wait that is implied by the
    instruction's program-order context plus its other waits.  It raises if a
    Matmult still carries more than one wait afterwards (compile would fail).
    """
    import bisect
    from collections import defaultdict

    all_insts = []
    for f in nc.m.functions:
        for bb in f.blocks:
            all_insts.extend(bb.instructions)

    # cumulative semaphore values in scheduled order; producers[s] maps the
    # running total to the instruction index whose completion reaches it
    cum = defaultdict(int)
    producers = defaultdict(list)  # sem id -> [(cum_value, idx)]
    inst_updates = defaultdict(list)  # idx -> [(sem, cum_value_at_completion)]
    poisoned = set()
    for idx, inst in enumerate(all_insts):
        si = inst.sync_info
        if si is None:
            continue
        for u in si.on_update or []:
            if u.update_mode != "sem-inc" or u.update_reg is not None:
                poisoned.add(u.id)
                continue
            cum[u.id] += u.update_value
            producers[u.id].append((cum[u.id], idx))
            inst_updates[idx].append((u.id, cum[u.id]))

    def producer_idx(sem, val):
        lst = producers.get(sem)
        if not lst:
            return None
        vals = [c for c, _ in lst]
        i = bisect.bisect_left(vals, val)
        return lst[i][1] if i < len(vals) else None

    prev_idx = [None] * len(all_insts)
    last_on_engine = {}
    for idx, inst in enumerate(all_insts):
        e = inst.engine
        prev_idx[idx] = last_on_engine.get(e)
        last_on_engine[e] = idx

    def usable_waits(inst):
        si = inst.sync_info
        if si is None:
            return []
        return [
            w
            for w in si.on_wait or []
            if w.sync_type == "semaphore"
            and w.wait_mode == "sem-ge-imm"
            and w.wait_reg is None
            and w.id not in poisoned
        ]

    issue_vc = {}
    comp_vc = {}

    def merge(dst, src):
        for k, v in src.items():
            if dst.get(k, -1) < v:
                dst[k] = v

    def is_async_dispatch(inst):
        # HWDGE/SWDGE DMA waits are enforced in the DMA ring, not by the
        # issuing sequencer — successors on the same engine do NOT
        # happen-after them, so their waits must not leak into program-order
        # context.
        return type(inst).__name__ in ("InstDMACopy", "InstTriggeredCopy")

    def get_issue_vc(idx):
        # context known when instruction idx ISSUES: previous same-engine
        # instruction's issue context (NOT its async updates) plus the
        # producers' completion clocks of this instruction's own waits
        if idx in issue_vc:
            return issue_vc[idx]
        issue_vc[idx] = {}  # cycle guard: conservative empty
        vc = {}
        p = prev_idx[idx]
        while p is not None and is_async_dispatch(all_insts[p]):
            p = prev_idx[p]  # skip async-dispatch instructions' contexts
        if p is not None:
            merge(vc, get_issue_vc(p))
        for w in usable_waits(all_insts[idx]):
            pi = producer_idx(w.id, w.wait_value)
            if pi is not None:
                merge(vc, get_comp_vc(pi))
            if vc.get(w.id, -1) < w.wait_value:
                vc[w.id] = w.wait_value
        issue_vc[idx] = vc
        return vc

    def get_comp_vc(idx):
        if idx in comp_vc:
            return comp_vc[idx]
        comp_vc[idx] = {}  # cycle guard
        vc = dict(get_issue_vc(idx))
        for sem, val in inst_updates.get(idx, []):
            if vc.get(sem, -1) < val:
                vc[sem] = val
        comp_vc[idx] = vc
        return vc

    stripped = 0
    for idx, inst in enumerate(all_insts):
        si = inst.sync_info
        waits = list(si.on_wait or []) if si else []
        if len(waits) <= 1:
            continue
        keep = list(waits)
        for w in list(keep):
            if (
                w.sync_type != "semaphore"
                or w.wait_mode != "sem-ge-imm"
                or w.wait_reg is not None
                or w.id in poisoned
            ):
                continue
            ctx = {}
            p = prev_idx[idx]
            if p is not None:
                merge(ctx, get_issue_vc(p))
            for w2 in keep:
                if w2 is w:
                    continue
                pi = producer_idx(w2.id, w2.wait_value)
                if pi is not None:
                    merge(ctx, get_comp_vc(pi))
                if ctx.get(w2.id, -1) < w2.wait_value:
                    ctx[w2.id] = w2.wait_value
            if ctx.get(w.id, -1) >= w.wait_value:
                keep.remove(w)
                stripped += 1
        if len(keep) != len(waits):
            si.on_wait = keep

    # TPB compute instructions encode exactly ONE wait slot (one
    # NEURON_ISA_TPB_EVENTS field per struct); only DMA instructions may carry
    # more.  Split any surviving extra waits into standalone EventSemaphore
    # instructions on the same engine queue immediately before the owner.
    ev_n = 0
    for f in nc.m.functions:
        for bb in f.blocks:
            insts = list(bb.instructions)
            out = []
            changed = False
            for inst in insts:
                si = inst.sync_info
                waits = list(si.on_wait or []) if si else []
                if len(waits) > 1:
                    for w in waits[:-1]:
                        ev_n += 1
                        ev = mybir.InstEventSemaphore(
                            name=f"evsplit-{ev_n}",
                            engine=inst.engine,
                            sync_info=mybir.SyncInfo(on_wait=[w], on_update=[]),
                        )
                        out.append(ev)
                        changed = True
                    si.on_wait = [waits[-1]]
                out.append(inst)
            if changed:
                bb.instructions = out


def _get_nc() -> bass.Bass:
    if "nc" not in _NC_CACHE:
        _NC_CACHE["nc"] = _build_nc()
    return _NC_CACHE["nc"]


def kernel(x: np.ndarray, reference_points: np.ndarray) -> np.ndarray:
    x = np.ascontiguousarray(x, dtype=np.float32)
    r = np.ascontiguousarray(reference_points, dtype=np.float32)
    assert x.shape == (N, D) and r.shape == (M, D)

    nc = _get_nc()
    in_maps = [
        {"x": x[c * NSHARD : (c + 1) * NSHARD], "reference_points": r}
        for c in range(NCORES)
    ]
    # The exec unit is occasionally found wedged (NRT_EXEC_UNIT_UNRECOVERABLE,
    # left over from an earlier process). Observed recoveries always came from
    # a FRESH client connection, so between attempts also tear down the jax
    # backend (re-registers on next use) in addition to backing off.
    import time as _time

    for backoff in (30.0, 90.0, 180.0, None):
        try:
            res = run_bass_kernel_spmd(nc, in_maps, list(range(NCORES)))
            break
        except Exception:  # noqa: BLE001 - device-transient errors
            if backoff is None:
                raise
            _time.sleep(backoff)
            try:
                import jax.extend.backend as _jeb

                _jeb.clear_backends()
            except Exception:  # noqa: BLE001 - best-effort reset
                pass
    full = np.concatenate([res.results[c]["out"] for c in range(NCORES)], axis=0)
    return full.astype(np.float32)

